# revision 10
# baseline (speedup 1.0000x reference)
"""Multi-head causal self-attention (B=4, N=2048, D=1024, H=16) on 8 TRN2 cores.

Sharding: 8 cores = 4 batches x 2 head-groups (8 heads / 512 dims each).
Per core (batch b, group g):
  - QKV projections computed in transposed layout (dims on partitions):
      Q^T, K^T = W^T-chunks (lhsT) x x^T (rhs), accumulated over 8 din chunks.
      V computed in natural [token, dv] layout (lhsT = x^T chunk).
  - Attention computed as S^T tiles [keys(128) x queries(512)] so that
    exp(S) feeds the P^T.V matmul directly (contraction over keys on
    partitions, no transposes anywhere). Softmax denominators come from a
    ones-column appended to V (row HD of the PV accumulator), normalization
    is folded into the PSUM->SBUF eviction. Causal masking = skip blocks
    above the diagonal + multiply the <=3 diagonal-region tiles by
    precomputed 0/1 masks after exp. No max-subtraction: scores are ~N(0,1)
    after the 1/sqrt(hd) scale, exp is safe in fp32.
  - O-projection partial: attnT (lhsT) x Wo-slice (rhs) -> [2048, 1024]
    partial output per core; host sums the two group partials per batch.

Matmuls use float32r (full fp32 storage, full PE rate at free dim >= 256).
"""

import numpy as np

import concourse.bass as bass
import concourse.tile as tile
from concourse import bacc, mybir
from concourse import bass_utils
from concourse._compat import with_exitstack
from concourse.bass import ts, ds

B, N, D, H, HD = 4, 2048, 1024, 16, 64
GROUPS = 2              # head groups (cores per batch)
DC = D // GROUPS        # 512 dims per core
HPC = H // GROUPS       # 8 heads per core
P = 128
QW = 512                # query strip width / matmul free dim
NDIN = D // P           # 8 contraction chunks for QKV
NSTRIP = DC // P        # 4 dq strips per core (2 heads each)
NTT = N // P            # 16 token tiles
NTS = N // QW           # 4 token strips
NQB = QW // P           # 4 query blocks per strip

F32 = mybir.dt.float32
F32R = mybir.dt.float32r  # matmul operands: fp32 storage, ~tf32 matmul precision, full PE rate


def _emit(ctx, tc, xT, wq, wk, wv, wo, bq, bk, bv, masks, out):
    nc = tc.nc
    EXP = mybir.ActivationFunctionType.Exp

    const = ctx.enter_context(tc.tile_pool(name="const", bufs=1))
    p_mm = ctx.enter_context(tc.tile_pool(name="p_mm", bufs=3, space="PSUM"))
    p_st = ctx.enter_context(tc.tile_pool(name="p_st", bufs=3, space="PSUM"))
    p_pv = ctx.enter_context(tc.tile_pool(name="p_pv", bufs=2, space="PSUM"))
    p_pt = ctx.enter_context(tc.tile_pool(name="p_pt", bufs=3))
    p_small = ctx.enter_context(tc.tile_pool(name="p_small", bufs=2))
    p_dram = ctx.enter_context(tc.tile_pool(name="p_dram", bufs=2, space="DRAM"))

    # constants: masks [128, 4, 512], per-strip biases [128, 4], bv broadcast
    maskt = const.tile([P, NQB, QW], F32)
    nc.sync.dma_start(out=maskt, in_=masks.rearrange("m p q -> p m q"))
    bqt = const.tile([P, NSTRIP], F32)
    nc.sync.dma_start(out=bqt, in_=bq.rearrange("(s p) -> p s", p=P))
    bkt = const.tile([P, NSTRIP], F32)
    nc.sync.dma_start(out=bkt, in_=bk.rearrange("(s p) -> p s", p=P))
    bvb = const.tile([P, DC], F32)
    nc.sync.dma_start(out=bvb, in_=bv.unsqueeze(0).partition_broadcast(P))

    # persistent per-batch tensors
    attnT = const.tile([P, NSTRIP, N], F32R)                # normalized attn^T
    vplus = const.tile([P, NTT, HPC, HD + 1], F32R)         # V | ones column
    # memset on an f32r tile is invalid ISA; write the ones column via a DVE
    # copy from an f32 staging tile (a valid f32r-rounding producer)
    ones_f32 = const.tile([P, NTT * HPC], F32)
    nc.vector.memset(ones_f32, 1.0)
    nc.vector.tensor_copy(
        out=vplus[:, :, :, HD:HD + 1],
        in_=ones_f32.rearrange("p (a b) -> p a b", b=HPC).unsqueeze(3),
    )

    wqr = wq.rearrange("(c p) f -> c p f", p=P)
    wkr = wk.rearrange("(c p) f -> c p f", p=P)
    wvr = wv.rearrange("(c p) f -> c p f", p=P)
    xTr = xT.rearrange("(c p) n -> c p n", p=P)

    with tc.tile_pool(name="p_xt", bufs=1) as p_xt:
        xt = p_xt.tile([P, NDIN, N], F32R)          # x^T resident, 64KB/part
        for c in range(NDIN):
            nc.sync.dma_start(out=xt[:, c, :], in_=xTr[c])

        # ---- phase A0: V = x @ Wv + bv (natural layout), all 8 heads ----
        with tc.tile_pool(name="p_wv", bufs=1) as p_wv:
            wvt = p_wv.tile([P, NDIN, DC], F32R)
            for c in range(NDIN):
                nc.sync.dma_start(out=wvt[:, c, :], in_=wvr[c])
            for tt in range(NTT):
                psv = p_mm.tile([P, DC], F32, tag="mm")
                for c in range(NDIN):
                    nc.tensor.matmul(
                        psv, lhsT=(xt[:, c, ts(tt, P)]), rhs=(wvt[:, c, :]),
                        start=(c == 0), stop=(c == NDIN - 1),
                    )
                for h in range(HPC):
                    nc.vector.tensor_add(
                        out=vplus[:, tt, h, 0:HD],
                        in0=psv[:, ts(h, HD)], in1=bvb[:, ts(h, HD)],
                    )

        # ---- per dq-strip: Q^T/K^T projection then attention for 2 heads ----
        with (
            tc.tile_pool(name="p_w", bufs=2) as p_w,
            tc.tile_pool(name="p_qk", bufs=1) as p_qk,
        ):
            for s in range(NSTRIP):
                wqs = p_w.tile([P, NDIN, P], F32R, tag="wq")
                wks = p_w.tile([P, NDIN, P], F32R, tag="wk")
                for c in range(NDIN):
                    nc.sync.dma_start(out=wqs[:, c, :], in_=wqr[c, :, ts(s, P)])
                    nc.sync.dma_start(out=wks[:, c, :], in_=wkr[c, :, ts(s, P)])
                qts = p_qk.tile([P, N], F32R, tag="qt")
                kts = p_qk.tile([P, N], F32R, tag="kt")
                for t in range(NTS):
                    psq = p_mm.tile([P, QW], F32, tag="mm")
                    for c in range(NDIN):
                        nc.tensor.matmul(
                            psq, lhsT=(wqs[:, c, :]), rhs=(xt[:, c, ts(t, QW)]),
                            start=(c == 0), stop=(c == NDIN - 1),
                        )
                    nc.vector.tensor_scalar_add(
                        out=qts[:, ts(t, QW)], in0=psq, scalar1=bqt[:, s:s + 1])
                    psk = p_mm.tile([P, QW], F32, tag="mm")
                    for c in range(NDIN):
                        nc.tensor.matmul(
                            psk, lhsT=(wks[:, c, :]), rhs=(xt[:, c, ts(t, QW)]),
                            start=(c == 0), stop=(c == NDIN - 1),
                        )
                    nc.vector.tensor_scalar_add(
                        out=kts[:, ts(t, QW)], in0=psk, scalar1=bkt[:, s:s + 1])

                # attention for the two heads living in this strip
                for h2 in range(2):
                    po = h2 * HD
                    h = 2 * s + h2
                    for qs in range(NTS):
                        nkc = NQB * qs + NQB      # causal: key blocks 0..nkc-1
                        pvp = p_pv.tile([HD + 1, QW], F32, tag="pv")
                        pts = {}
                        LOOK = 2
                        for i in range(nkc + LOOK):
                            if i < nkc:
                                kc = i
                                pst = p_st.tile([P, QW], F32, tag="st")
                                nc.tensor.matmul(
                                    pst,
                                    lhsT=(kts[po:po + HD, ts(kc, P)]),
                                    rhs=(qts[po:po + HD, ts(qs, QW)]),
                                    start=True, stop=True,
                                )
                                pt = p_pt.tile([P, QW], F32R, tag="pt")
                                nc.scalar.activation(
                                    out=pt, in_=pst, func=EXP, scale=0.125)
                                if kc >= NQB * qs:
                                    nc.vector.tensor_mul(
                                        pt, pt, maskt[:, kc - NQB * qs, :])
                                pts[kc] = pt
                            if i >= LOOK:
                                kc = i - LOOK
                                nc.tensor.matmul(
                                    pvp, lhsT=(vplus[:, kc, h, :]),
                                    rhs=(pts.pop(kc)),
                                    start=(kc == 0), stop=(kc == nkc - 1),
                                )
                        recip = p_small.tile([1, QW], F32, tag="recip")
                        nc.vector.reciprocal(out=recip, in_=pvp[HD:HD + 1, :])
                        # broadcast across partitions via a DRAM round-trip
                        # (SBUF-source partition-broadcast DMA is rejected)
                        recip_d = p_dram.tile([1, QW], F32, tag="recipd")
                        nc.sync.dma_start(out=recip_d, in_=recip)
                        rb = p_small.tile([HD, QW], F32, tag="rb")
                        nc.sync.dma_start(out=rb, in_=recip_d.partition_broadcast(HD))
                        nc.vector.tensor_mul(
                            out=attnT[po:po + HD, s, ts(qs, QW)],
                            in0=pvp[0:HD, :], in1=rb,
                        )

    # ---- phase C: partial output = attnT^T @ Wo_slice ----
    wor = wo.rearrange("(c p) f -> c p f", p=P)
    with (
        tc.tile_pool(name="p_wo", bufs=1) as p_wo,
        tc.tile_pool(name="p_osb", bufs=3) as p_osb,
    ):
        wot = p_wo.tile([P, NSTRIP, D], F32R)
        for c in range(NSTRIP):
            nc.sync.dma_start(out=wot[:, c, :], in_=wor[c])
        for tt in range(NTT):
            osb = p_osb.tile([P, D], F32, tag="osb")
            for half in range(2):
                pso = p_mm.tile([P, QW], F32, tag="mm")
                for c in range(NSTRIP):
                    nc.tensor.matmul(
                        pso, lhsT=(attnT[:, c, ts(tt, P)]),
                        rhs=(wot[:, c, ds(half * QW, QW)]),
                        start=(c == 0), stop=(c == NSTRIP - 1),
                    )
                nc.vector.tensor_copy(out=osb[:, ds(half * QW, QW)], in_=pso)
            nc.sync.dma_start(out=out[ts(tt, P), :], in_=osb)


_emit_wrapped = with_exitstack(_emit)

_NC_CACHE = None


def _build():
    global _NC_CACHE
    if _NC_CACHE is not None:
        return _NC_CACHE
    nc = bacc.Bacc("TRN2", target_bir_lowering=False, debug=False)
    xT = nc.dram_tensor("xt", [D, N], F32R, kind="ExternalInput").ap()
    wq = nc.dram_tensor("wq", [D, DC], F32R, kind="ExternalInput").ap()
    wk = nc.dram_tensor("wk", [D, DC], F32R, kind="ExternalInput").ap()
    wv = nc.dram_tensor("wv", [D, DC], F32R, kind="ExternalInput").ap()
    wo = nc.dram_tensor("wo", [DC, D], F32R, kind="ExternalInput").ap()
    bq = nc.dram_tensor("bq", [DC], F32, kind="ExternalInput").ap()
    bk = nc.dram_tensor("bk", [DC], F32, kind="ExternalInput").ap()
    bv = nc.dram_tensor("bv", [DC], F32, kind="ExternalInput").ap()
    masks = nc.dram_tensor("masks", [NQB, P, QW], F32, kind="ExternalInput").ap()
    out = nc.dram_tensor("out", [N, D], F32, kind="ExternalOutput").ap()
    with tile.TileContext(nc) as tc:
        _emit_wrapped(tc, xT, wq, wk, wv, wo, bq, bk, bv, masks, out)
    nc.compile()
    _NC_CACHE = nc
    return nc


def _make_masks():
    # masks[j] applies to the S^T tile whose key block sits j query-blocks
    # into the diagonal 512-wide region: [keys(128) x queries(512)].
    m = np.zeros((NQB, P, QW), np.float32)
    tri = np.triu(np.ones((P, P), np.float32))  # key <= query kept
    for j in range(NQB):
        for i in range(NQB):
            if i > j:
                m[j, :, i * P:(i + 1) * P] = 1.0
            elif i == j:
                m[j, :, i * P:(i + 1) * P] = tri
    return m


def _in_maps(x, Wq, bq, Wk, bk, Wv, bv, Wo):
    masks = _make_masks()
    maps = []
    for b in range(B):
        xt_b = np.ascontiguousarray(np.asarray(x[b]).T)
        for g in range(GROUPS):
            sl = slice(g * DC, (g + 1) * DC)
            maps.append({
                "xt": xt_b,
                "wq": np.ascontiguousarray(Wq[:, sl]),
                "wk": np.ascontiguousarray(Wk[:, sl]),
                "wv": np.ascontiguousarray(Wv[:, sl]),
                "wo": np.ascontiguousarray(Wo[sl, :]),
                "bq": np.ascontiguousarray(bq[sl]),
                "bk": np.ascontiguousarray(bk[sl]),
                "bv": np.ascontiguousarray(bv[sl]),
                "masks": masks,
            })
    return maps


def run(inputs, trace=False, tmpdir=None):
    """Build+run on 8 cores. Returns (out [B,N,D] f32, BassKernelResults)."""
    x = np.asarray(inputs["x"], np.float32)
    args = [np.asarray(inputs[k], np.float32) for k in
            ("Wq", "bq", "Wk", "bk", "Wv", "bv", "Wo")]
    bo = np.asarray(inputs["bo"], np.float32)
    nc = _build()
    maps = _in_maps(x, *args)
    if trace:
        bass_utils.upload_artifacts = lambda d: d
    res = bass_utils.run_bass_kernel_spmd(
        nc, maps, core_ids=list(range(8)), trace=trace, tmpdir=tmpdir)
    out = np.empty((B, N, D), np.float32)
    for b in range(B):
        out[b] = res.results[2 * b]["out"] + res.results[2 * b + 1]["out"] + bo
    return out, res


def kernel(**inputs):
    out, _ = run(inputs)
    return out


# revision 16
# speedup vs baseline: 1.5298x; 1.5298x over previous
"""Multi-head causal self-attention (B=4, N=2048, D=1024, H=16) on 8 TRN2 cores.

Sharding: 8 cores = 4 batches x 2 head-groups (8 heads / 512 dims each).
Per core (batch b, group g):
  - QKV projections computed in transposed layout (dims on partitions):
      Q^T, K^T = W^T-chunks (lhsT) x x^T (rhs), accumulated over 8 din chunks.
      V computed in natural [token, dv] layout (lhsT = x^T chunk).
  - Attention computed as S^T tiles [keys(128) x queries(512)] so that
    exp(S) feeds the P^T.V matmul directly (contraction over keys on
    partitions, no transposes anywhere). Softmax denominators come from a
    ones-column appended to V (row HD of the PV accumulator), normalization
    is folded into the PSUM->SBUF eviction. Causal masking = skip blocks
    above the diagonal + multiply the <=3 diagonal-region tiles by
    precomputed 0/1 masks after exp. No max-subtraction: scores are ~N(0,1)
    after the 1/sqrt(hd) scale, exp is safe in fp32.
  - O-projection partial: attnT (lhsT) x Wo-slice (rhs) -> [2048, 1024]
    partial output per core; host sums the two group partials per batch.

Matmuls use float32r (full fp32 storage, full PE rate at free dim >= 256).
"""

import numpy as np

import concourse.bass as bass
import concourse.tile as tile
from concourse import bacc, mybir
from concourse import bass_utils
from concourse._compat import with_exitstack
from concourse.bass import ts, ds

B, N, D, H, HD = 4, 2048, 1024, 16, 64
GROUPS = 2              # head groups (cores per batch)
DC = D // GROUPS        # 512 dims per core
HPC = H // GROUPS       # 8 heads per core
P = 128
QW = 512                # query strip width / matmul free dim
NDIN = D // P           # 8 contraction chunks for QKV
NSTRIP = DC // P        # 4 dq strips per core (2 heads each)
NTT = N // P            # 16 token tiles
NTS = N // QW           # 4 token strips
NQB = QW // P           # 4 query blocks per strip

F32 = mybir.dt.float32
F32R = mybir.dt.float32r  # matmul operands: fp32 storage, ~tf32 matmul precision, full PE rate


def _emit(ctx, tc, xT, wq, wk, wv, wo, bq, bk, bv, masks, out):
    nc = tc.nc
    EXP = mybir.ActivationFunctionType.Exp

    const = ctx.enter_context(tc.tile_pool(name="const", bufs=1))
    p_mm = ctx.enter_context(tc.tile_pool(name="p_mm", bufs=2, space="PSUM"))
    p_st = ctx.enter_context(tc.tile_pool(name="p_st", bufs=2, space="PSUM"))
    p_pv = ctx.enter_context(tc.tile_pool(name="p_pv", bufs=2, space="PSUM"))
    p_pt = ctx.enter_context(tc.tile_pool(name="p_pt", bufs=3))
    p_small = ctx.enter_context(tc.tile_pool(name="p_small", bufs=2))
    p_dram = ctx.enter_context(tc.tile_pool(name="p_dram", bufs=2, space="DRAM"))

    # constants: masks [128, 4, 512], per-strip biases [128, 4], bv broadcast
    maskt = const.tile([P, NQB, QW], F32)
    nc.sync.dma_start(out=maskt, in_=masks.rearrange("m p q -> p m q"))
    bqt = const.tile([P, NSTRIP], F32)
    nc.sync.dma_start(out=bqt, in_=bq.rearrange("(s p) -> p s", p=P))
    bkt = const.tile([P, NSTRIP], F32)
    nc.sync.dma_start(out=bkt, in_=bk.rearrange("(s p) -> p s", p=P))
    bvb = const.tile([P, DC], F32)
    nc.sync.dma_start(out=bvb, in_=bv.unsqueeze(0).partition_broadcast(P))

    # persistent per-batch tensors
    attnT = const.tile([P, NSTRIP, N], F32R)                # normalized attn^T
    vplus = const.tile([P, NTT, HPC, HD + 1], F32R)         # V | ones column
    # memset on an f32r tile is invalid ISA; write the ones column via a DVE
    # copy from an f32 staging tile (a valid f32r-rounding producer)
    ones_f32 = const.tile([P, NTT * HPC], F32)
    nc.vector.memset(ones_f32, 1.0)
    nc.vector.tensor_copy(
        out=vplus[:, :, :, HD:HD + 1],
        in_=ones_f32.rearrange("p (a b) -> p a b", b=HPC).unsqueeze(3),
    )

    wqr = wq.rearrange("(c p) f -> c p f", p=P)
    wkr = wk.rearrange("(c p) f -> c p f", p=P)
    wvr = wv.rearrange("(c p) f -> c p f", p=P)
    xTr = xT.rearrange("(c p) n -> c p n", p=P)

    with tc.tile_pool(name="p_xt", bufs=1) as p_xt:
        xt = p_xt.tile([P, NDIN, N], F32R)          # x^T resident, 64KB/part
        for c in range(NDIN):
            nc.sync.dma_start(out=xt[:, c, :], in_=xTr[c])

        # ---- phase A0: V = x @ Wv + bv (natural layout), all 8 heads ----
        with tc.tile_pool(name="p_wv", bufs=1) as p_wv:
            wvt = p_wv.tile([P, NDIN, DC], F32R)
            for c in range(NDIN):
                nc.sync.dma_start(out=wvt[:, c, :], in_=wvr[c])
            for tt in range(NTT):
                psv = p_mm.tile([P, DC], F32, tag="mm")
                for c in range(NDIN):
                    nc.tensor.matmul(
                        psv, lhsT=(xt[:, c, ts(tt, P)]), rhs=(wvt[:, c, :]),
                        start=(c == 0), stop=(c == NDIN - 1),
                    )
                nc.vector.tensor_add(
                    out=vplus[:, tt, :, 0:HD],
                    in0=psv.rearrange("p (h d) -> p h d", d=HD),
                    in1=bvb.rearrange("p (h d) -> p h d", d=HD),
                )

        # ---- per dq-strip: Q^T/K^T projection then attention for 2 heads ----
        with (
            tc.tile_pool(name="p_w", bufs=2) as p_w,
            tc.tile_pool(name="p_qk", bufs=1) as p_qk,
        ):
            for s in range(NSTRIP):
                wqs = p_w.tile([P, NDIN, P], F32R, tag="wq")
                wks = p_w.tile([P, NDIN, P], F32R, tag="wk")
                for c in range(NDIN):
                    nc.sync.dma_start(out=wqs[:, c, :], in_=wqr[c, :, ts(s, P)])
                    nc.sync.dma_start(out=wks[:, c, :], in_=wkr[c, :, ts(s, P)])
                qts = p_qk.tile([P, N], F32R, tag="qt")
                kts = p_qk.tile([P, N], F32R, tag="kt")
                for t in range(NTS):
                    psq = p_mm.tile([P, QW], F32, tag="mm")
                    for c in range(NDIN):
                        nc.tensor.matmul(
                            psq, lhsT=(wqs[:, c, :]), rhs=(xt[:, c, ts(t, QW)]),
                            start=(c == 0), stop=(c == NDIN - 1),
                        )
                    nc.vector.tensor_scalar_add(
                        out=qts[:, ts(t, QW)], in0=psq, scalar1=bqt[:, s:s + 1])
                    psk = p_mm.tile([P, QW], F32, tag="mm")
                    for c in range(NDIN):
                        nc.tensor.matmul(
                            psk, lhsT=(wks[:, c, :]), rhs=(xt[:, c, ts(t, QW)]),
                            start=(c == 0), stop=(c == NDIN - 1),
                        )
                    nc.vector.tensor_scalar_add(
                        out=kts[:, ts(t, QW)], in0=psk, scalar1=bkt[:, s:s + 1])

                # attention for the two heads living in this strip.
                # Per (head, query strip): S^T in kc-pairs (one batched exp per
                # pair), PV accumulation, then UNNORMALIZED eviction; softmax
                # sums rows are collected in sums_sb and normalized in one
                # batched reciprocal per strip (a [1,512] DVE reciprocal costs
                # ~4us serialized — batching across partitions makes it one op).
                # sums rows live at partition offsets {0,32,64,96} x 2 column
                # blocks (DVE partition offsets must be 32-aligned); unused
                # rows are memset to 1.0 so the batched reciprocal stays finite
                sums_sb = p_small.tile([P, 2, QW], F32, tag="sums")
                nc.vector.memset(sums_sb, 1.0)
                for h2 in range(2):
                    po = h2 * HD
                    h = 2 * s + h2
                    for qs in range(NTS):
                        nkc = NQB * qs + NQB      # causal: key blocks 0..nkc-1
                        npair = nkc // 2
                        pvp = p_pv.tile([HD + 1, QW], F32, tag="pv")
                        pts = {}
                        LOOKP = 1
                        for ip in range(npair + LOOKP):
                            if ip < npair:
                                pst = p_st.tile([P, 2, QW], F32, tag="st")
                                for j2 in range(2):
                                    kc = 2 * ip + j2
                                    nc.tensor.matmul(
                                        pst[:, j2, :],
                                        lhsT=(kts[po:po + HD, ts(kc, P)]),
                                        rhs=(qts[po:po + HD, ts(qs, QW)]),
                                        start=True, stop=True,
                                    )
                                pt = p_pt.tile([P, 2, QW], F32R, tag="pt")
                                nc.scalar.activation(
                                    out=pt, in_=pst, func=EXP, scale=0.125)
                                for j2 in range(2):
                                    kc = 2 * ip + j2
                                    if kc >= NQB * qs:
                                        nc.vector.tensor_mul(
                                            pt[:, j2, :], pt[:, j2, :],
                                            maskt[:, kc - NQB * qs, :])
                                pts[ip] = pt
                            if ip >= LOOKP:
                                jp = ip - LOOKP
                                pt = pts.pop(jp)
                                for j2 in range(2):
                                    kc = 2 * jp + j2
                                    nc.tensor.matmul(
                                        pvp, lhsT=(vplus[:, kc, h, :]),
                                        rhs=(pt[:, j2, :]),
                                        start=(kc == 0), stop=(kc == nkc - 1),
                                    )
                        nc.vector.tensor_copy(
                            out=sums_sb[32 * qs:32 * qs + 1, h2, :],
                            in_=pvp[HD:HD + 1, :])
                        nc.vector.tensor_copy(
                            out=attnT[po:po + HD, s, ts(qs, QW)], in_=pvp[0:HD, :])
                # batched normalization for the whole strip (8 groups at once)
                recip_sb = p_small.tile([P, 2, QW], F32, tag="recip")
                nc.vector.reciprocal(out=recip_sb, in_=sums_sb)
                # broadcast across partitions via a DRAM round-trip
                # (SBUF-source partition-broadcast DMA is rejected)
                recip_d = p_dram.tile([NTS, 2, QW], F32, tag="recipd")
                nc.sync.dma_start(
                    out=recip_d,
                    in_=recip_sb.rearrange("(a b) c f -> a b c f", b=32)[:, 0, :, :])
                for h2 in range(2):
                    po = h2 * HD
                    for qs in range(NTS):
                        # full-128-partition broadcast so rb[po:po+HD] shares
                        # the base partition with the attnT slice (DVE rule)
                        rb = p_small.tile([P, QW], F32, tag="rb", bufs=3)
                        nc.sync.dma_start(
                            out=rb,
                            in_=recip_d[qs, h2, :].unsqueeze(0)
                            .partition_broadcast(P))
                        sl = attnT[po:po + HD, s, ts(qs, QW)]
                        nc.vector.tensor_mul(out=sl, in0=sl, in1=rb[po:po + HD, :])

    # ---- phase C: partial output = attnT^T @ Wo_slice ----
    wor = wo.rearrange("(c p) f -> c p f", p=P)
    with (
        tc.tile_pool(name="p_wo", bufs=1) as p_wo,
        tc.tile_pool(name="p_osb", bufs=3) as p_osb,
    ):
        wot = p_wo.tile([P, NSTRIP, D], F32R)
        for c in range(NSTRIP):
            nc.sync.dma_start(out=wot[:, c, :], in_=wor[c])
        for tt in range(NTT):
            osb = p_osb.tile([P, D], F32, tag="osb")
            for half in range(2):
                pso = p_mm.tile([P, QW], F32, tag="mm")
                for c in range(NSTRIP):
                    nc.tensor.matmul(
                        pso, lhsT=(attnT[:, c, ts(tt, P)]),
                        rhs=(wot[:, c, ds(half * QW, QW)]),
                        start=(c == 0), stop=(c == NSTRIP - 1),
                    )
                nc.vector.tensor_copy(out=osb[:, ds(half * QW, QW)], in_=pso)
            nc.sync.dma_start(out=out[ts(tt, P), :], in_=osb)


_emit_wrapped = with_exitstack(_emit)

_NC_CACHE = None


def _build():
    global _NC_CACHE
    if _NC_CACHE is not None:
        return _NC_CACHE
    nc = bacc.Bacc("TRN2", target_bir_lowering=False, debug=False)
    xT = nc.dram_tensor("xt", [D, N], F32R, kind="ExternalInput").ap()
    wq = nc.dram_tensor("wq", [D, DC], F32R, kind="ExternalInput").ap()
    wk = nc.dram_tensor("wk", [D, DC], F32R, kind="ExternalInput").ap()
    wv = nc.dram_tensor("wv", [D, DC], F32R, kind="ExternalInput").ap()
    wo = nc.dram_tensor("wo", [DC, D], F32R, kind="ExternalInput").ap()
    bq = nc.dram_tensor("bq", [DC], F32, kind="ExternalInput").ap()
    bk = nc.dram_tensor("bk", [DC], F32, kind="ExternalInput").ap()
    bv = nc.dram_tensor("bv", [DC], F32, kind="ExternalInput").ap()
    masks = nc.dram_tensor("masks", [NQB, P, QW], F32, kind="ExternalInput").ap()
    out = nc.dram_tensor("out", [N, D], F32, kind="ExternalOutput").ap()
    with tile.TileContext(nc) as tc:
        _emit_wrapped(tc, xT, wq, wk, wv, wo, bq, bk, bv, masks, out)
    nc.compile()
    _NC_CACHE = nc
    return nc


def _make_masks():
    # masks[j] applies to the S^T tile whose key block sits j query-blocks
    # into the diagonal 512-wide region: [keys(128) x queries(512)].
    m = np.zeros((NQB, P, QW), np.float32)
    tri = np.triu(np.ones((P, P), np.float32))  # key <= query kept
    for j in range(NQB):
        for i in range(NQB):
            if i > j:
                m[j, :, i * P:(i + 1) * P] = 1.0
            elif i == j:
                m[j, :, i * P:(i + 1) * P] = tri
    return m


def _in_maps(x, Wq, bq, Wk, bk, Wv, bv, Wo):
    masks = _make_masks()
    maps = []
    for b in range(B):
        xt_b = np.ascontiguousarray(np.asarray(x[b]).T)
        for g in range(GROUPS):
            sl = slice(g * DC, (g + 1) * DC)
            maps.append({
                "xt": xt_b,
                "wq": np.ascontiguousarray(Wq[:, sl]),
                "wk": np.ascontiguousarray(Wk[:, sl]),
                "wv": np.ascontiguousarray(Wv[:, sl]),
                "wo": np.ascontiguousarray(Wo[sl, :]),
                "bq": np.ascontiguousarray(bq[sl]),
                "bk": np.ascontiguousarray(bk[sl]),
                "bv": np.ascontiguousarray(bv[sl]),
                "masks": masks,
            })
    return maps


def run(inputs, trace=False, tmpdir=None):
    """Build+run on 8 cores. Returns (out [B,N,D] f32, BassKernelResults)."""
    x = np.asarray(inputs["x"], np.float32)
    args = [np.asarray(inputs[k], np.float32) for k in
            ("Wq", "bq", "Wk", "bk", "Wv", "bv", "Wo")]
    bo = np.asarray(inputs["bo"], np.float32)
    nc = _build()
    maps = _in_maps(x, *args)
    if trace:
        bass_utils.upload_artifacts = lambda d: d
    res = bass_utils.run_bass_kernel_spmd(
        nc, maps, core_ids=list(range(8)), trace=trace, tmpdir=tmpdir)
    out = np.empty((B, N, D), np.float32)
    for b in range(B):
        out[b] = res.results[2 * b]["out"] + res.results[2 * b + 1]["out"] + bo
    return out, res


def kernel(**inputs):
    out, _ = run(inputs)
    return out


# revision 18
# speedup vs baseline: 1.5774x; 1.0311x over previous
"""Multi-head causal self-attention (B=4, N=2048, D=1024, H=16) on 8 TRN2 cores.

Sharding: 8 cores = 4 batches x 2 head-groups (8 heads / 512 dims each).
Per core (batch b, group g):
  - QKV projections computed in transposed layout (dims on partitions):
      Q^T, K^T = W^T-chunks (lhsT) x x^T (rhs), accumulated over 8 din chunks.
      V computed in natural [token, dv] layout (lhsT = x^T chunk).
  - Attention computed as S^T tiles [keys(128) x queries(512)] so that
    exp(S) feeds the P^T.V matmul directly (contraction over keys on
    partitions, no transposes anywhere). Softmax denominators come from a
    ones-column appended to V (row HD of the PV accumulator); normalization
    is deferred and batched per strip. Causal masking = skip blocks above
    the diagonal + multiply diagonal-region tiles by precomputed 0/1 masks
    after exp. No max-subtraction: scores are ~N(0,1) after the 1/sqrt(hd)
    scale, exp is safe in fp32.
  - O-projection partial: attnT (lhsT) x Wo-slice (rhs) -> [2048, 1024]
    partial output per core; host sums the two group partials per batch.

Dtypes: scores path float32r (fp32 storage, ~tf32 matmul precision, full PE
rate); P/V path bfloat16 (probs in [0,1], V ~N(0,1)).
"""

import numpy as np
import ml_dtypes

import concourse.bass as bass
import concourse.tile as tile
from concourse import bacc, mybir
from concourse import bass_utils
from concourse._compat import with_exitstack
from concourse.bass import ts, ds

B, N, D, H, HD = 4, 2048, 1024, 16, 64
GROUPS = 2              # head groups (cores per batch)
DC = D // GROUPS        # 512 dims per core
HPC = H // GROUPS       # 8 heads per core
P = 128
QW = 512                # query strip width / matmul free dim
NDIN = D // P           # 8 contraction chunks for QKV
NSTRIP = DC // P        # 4 dq strips per core (2 heads each)
NTT = N // P            # 16 token tiles
NTS = N // QW           # 4 token strips
NQB = QW // P           # 4 query blocks per strip

F32 = mybir.dt.float32
F32R = mybir.dt.float32r
BF16 = mybir.dt.bfloat16


def _emit(ctx, tc, xT, wq, wk, wv, wo, bq, bk, bv, masks, out):
    nc = tc.nc
    EXP = mybir.ActivationFunctionType.Exp

    const = ctx.enter_context(tc.tile_pool(name="const", bufs=1))
    p_mm = ctx.enter_context(tc.tile_pool(name="p_mm", bufs=2, space="PSUM"))
    p_pt = ctx.enter_context(tc.tile_pool(name="p_pt", bufs=3))
    p_small = ctx.enter_context(tc.tile_pool(name="p_small", bufs=2))
    p_dram = ctx.enter_context(tc.tile_pool(name="p_dram", bufs=2, space="DRAM"))

    # constants: masks [128, 4, 512], per-strip biases [128, 4], bv broadcast
    maskt = const.tile([P, NQB, QW], BF16)
    nc.sync.dma_start(out=maskt, in_=masks.rearrange("m p q -> p m q"))
    bqt = const.tile([P, NSTRIP], F32)
    nc.sync.dma_start(out=bqt, in_=bq.rearrange("(s p) -> p s", p=P))
    bkt = const.tile([P, NSTRIP], F32)
    nc.sync.dma_start(out=bkt, in_=bk.rearrange("(s p) -> p s", p=P))
    bvb = const.tile([P, DC], F32)
    nc.sync.dma_start(out=bvb, in_=bv.unsqueeze(0).partition_broadcast(P))

    # persistent per-batch tensors
    attnT = const.tile([P, NSTRIP, N], F32R)                # normalized attn^T
    vplus = const.tile([P, NTT, HPC, HD + 1], BF16)         # V | ones column
    # memset on an f32r/bf16 matmul-input tile is invalid ISA; write the ones
    # column via a DVE copy from an f32 staging tile (a valid rounding producer)
    ones_f32 = const.tile([P, NTT * HPC], F32)
    nc.vector.memset(ones_f32, 1.0)
    nc.vector.tensor_copy(
        out=vplus[:, :, :, HD:HD + 1],
        in_=ones_f32.rearrange("p (a b) -> p a b", b=HPC).unsqueeze(3),
    )

    wqr = wq.rearrange("(c p) f -> c p f", p=P)
    wkr = wk.rearrange("(c p) f -> c p f", p=P)
    wvr = wv.rearrange("(c p) f -> c p f", p=P)
    xTr = xT.rearrange("(c p) n -> c p n", p=P)

    with tc.tile_pool(name="p_xt", bufs=1) as p_xt:
        xt = p_xt.tile([P, NDIN, N], F32R)          # x^T resident, 64KB/part
        # token-strip-major loads so early-strip compute can start ASAP
        for t in range(NTS):
            for c in range(NDIN):
                nc.sync.dma_start(
                    out=xt[:, c, ts(t, QW)], in_=xTr[c, :, ts(t, QW)])

        with (
            tc.tile_pool(name="p_w", bufs=2) as p_w,
            tc.tile_pool(name="p_qk", bufs=1) as p_qk,
            tc.tile_pool(name="p_st", bufs=2, space="PSUM") as p_st,
            tc.tile_pool(name="p_pv", bufs=2, space="PSUM") as p_pv,
        ):
            def attn_group(s, h2, qs, qts, kts, sums_sb):
                """S^T/exp/PV for one (head, query strip); kc-paired."""
                po = h2 * HD
                h = 2 * s + h2
                nkc = NQB * qs + NQB          # causal: key blocks 0..nkc-1
                npair = nkc // 2
                pvp = p_pv.tile([HD + 1, QW], F32, tag="pv", name="pvp")
                pts = {}
                LOOKP = 1
                for ip in range(npair + LOOKP):
                    if ip < npair:
                        pst = p_st.tile([P, 2, QW], F32, tag="st", name="pst")
                        for j2 in range(2):
                            kc = 2 * ip + j2
                            nc.tensor.matmul(
                                pst[:, j2, :],
                                lhsT=kts[po:po + HD, ts(kc, P)],
                                rhs=qts[po:po + HD, ts(qs, QW)],
                                start=True, stop=True,
                            )
                        pt = p_pt.tile([P, 2, QW], BF16, tag="pt", name="pt")
                        nc.scalar.activation(out=pt, in_=pst, func=EXP, scale=0.125)
                        for j2 in range(2):
                            kc = 2 * ip + j2
                            if kc >= NQB * qs:
                                nc.vector.tensor_mul(
                                    pt[:, j2, :], pt[:, j2, :],
                                    maskt[:, kc - NQB * qs, :])
                        pts[ip] = pt
                    if ip >= LOOKP:
                        jp = ip - LOOKP
                        pt = pts.pop(jp)
                        for j2 in range(2):
                            kc = 2 * jp + j2
                            nc.tensor.matmul(
                                pvp, lhsT=vplus[:, kc, h, :], rhs=pt[:, j2, :],
                                start=(kc == 0), stop=(kc == nkc - 1),
                            )
                nc.vector.tensor_copy(
                    out=sums_sb[32 * qs:32 * qs + 1, h2, :],
                    in_=pvp[HD:HD + 1, :])
                nc.vector.tensor_copy(
                    out=attnT[po:po + HD, s, ts(qs, QW)], in_=pvp[0:HD, :])

            for s in range(NSTRIP):
                wqs = p_w.tile([P, NDIN, P], F32R, tag="wq")
                wks = p_w.tile([P, NDIN, P], F32R, tag="wk")
                for c in range(NDIN):
                    nc.sync.dma_start(out=wqs[:, c, :], in_=wqr[c, :, ts(s, P)])
                    nc.sync.dma_start(out=wks[:, c, :], in_=wkr[c, :, ts(s, P)])
                qts = p_qk.tile([P, N], F32R, tag="qt")
                kts = p_qk.tile([P, N], F32R, tag="kt")
                # sums rows at partition offsets {0,32,64,96} x 2 col blocks
                # (DVE partition offsets must be 32-aligned); unused rows are
                # memset to 1.0 so the batched reciprocal stays finite
                sums_sb = p_small.tile([P, 2, QW], F32, tag="sums")
                nc.vector.memset(sums_sb, 1.0)
                for t in range(NTS):
                    psq = p_mm.tile([P, QW], F32, tag="mm", name="psq")
                    for c in range(NDIN):
                        nc.tensor.matmul(
                            psq, lhsT=wqs[:, c, :], rhs=xt[:, c, ts(t, QW)],
                            start=(c == 0), stop=(c == NDIN - 1),
                        )
                    nc.vector.tensor_scalar_add(
                        out=qts[:, ts(t, QW)], in0=psq, scalar1=bqt[:, s:s + 1])
                    psk = p_mm.tile([P, QW], F32, tag="mm", name="psk")
                    for c in range(NDIN):
                        nc.tensor.matmul(
                            psk, lhsT=wks[:, c, :], rhs=xt[:, c, ts(t, QW)],
                            start=(c == 0), stop=(c == NDIN - 1),
                        )
                    nc.vector.tensor_scalar_add(
                        out=kts[:, ts(t, QW)], in0=psk, scalar1=bkt[:, s:s + 1])

                    if s == 0 and t == 0:
                        # V = x @ Wv + bv for all heads, nested here so the
                        # strip-0 attention (which needs early V tiles) can
                        # start as soon as possible
                        with tc.tile_pool(name="p_wv", bufs=1) as p_wv:
                            wvt = p_wv.tile([P, NDIN, DC], F32R)
                            for c in range(NDIN):
                                nc.sync.dma_start(out=wvt[:, c, :], in_=wvr[c])
                            for tt in range(NTT):
                                psv = p_mm.tile([P, DC], F32, tag="mm", name="psv")
                                for c in range(NDIN):
                                    nc.tensor.matmul(
                                        psv, lhsT=xt[:, c, ts(tt, P)],
                                        rhs=wvt[:, c, :],
                                        start=(c == 0), stop=(c == NDIN - 1),
                                    )
                                nc.vector.tensor_add(
                                    out=vplus[:, tt, :, 0:HD],
                                    in0=psv.rearrange("p (h d) -> p h d", d=HD),
                                    in1=bvb.rearrange("p (h d) -> p h d", d=HD),
                                )

                    # attention for both heads of this strip at query strip t
                    # (needs only K/Q token strips <= t, all already computed)
                    attn_group(s, 0, t, qts, kts, sums_sb)
                    attn_group(s, 1, t, qts, kts, sums_sb)

                # batched normalization for the whole strip (8 groups at once)
                recip_sb = p_small.tile([P, 2, QW], F32, tag="recip")
                nc.vector.reciprocal(out=recip_sb, in_=sums_sb)
                # broadcast across partitions via a DRAM round-trip
                # (SBUF-source partition-broadcast DMA is rejected)
                recip_d = p_dram.tile([NTS, 2, QW], F32, tag="recipd")
                nc.sync.dma_start(
                    out=recip_d,
                    in_=recip_sb.rearrange("(a b) c f -> a b c f", b=32)[:, 0, :, :])
                for h2 in range(2):
                    po = h2 * HD
                    for qs in range(NTS):
                        # full-128-partition broadcast so rb[po:po+HD] shares
                        # the base partition with the attnT slice (DVE rule)
                        rb = p_small.tile([P, QW], F32, tag="rb", bufs=3)
                        nc.sync.dma_start(
                            out=rb,
                            in_=recip_d[qs, h2, :].unsqueeze(0)
                            .partition_broadcast(P))
                        sl = attnT[po:po + HD, s, ts(qs, QW)]
                        nc.vector.tensor_mul(out=sl, in0=sl, in1=rb[po:po + HD, :])

    # ---- phase C: partial output = attnT^T @ Wo_slice ----
    wor = wo.rearrange("(c p) f -> c p f", p=P)
    with (
        tc.tile_pool(name="p_wo", bufs=1) as p_wo,
        tc.tile_pool(name="p_osb", bufs=3) as p_osb,
        tc.tile_pool(name="p_c", bufs=4, space="PSUM") as p_c,
    ):
        wot = p_wo.tile([P, NSTRIP, D], F32R)
        for c in range(NSTRIP):
            nc.sync.dma_start(out=wot[:, c, :], in_=wor[c])
        for tt in range(NTT):
            osb = p_osb.tile([P, D], F32, tag="osb")
            for half in range(2):
                pso = p_c.tile([P, QW], F32, tag="c", name="pso")
                for c in range(NSTRIP):
                    nc.tensor.matmul(
                        pso, lhsT=attnT[:, c, ts(tt, P)],
                        rhs=wot[:, c, ds(half * QW, QW)],
                        start=(c == 0), stop=(c == NSTRIP - 1),
                    )
                nc.vector.tensor_copy(out=osb[:, ds(half * QW, QW)], in_=pso)
            nc.sync.dma_start(out=out[ts(tt, P), :], in_=osb)


_emit_wrapped = with_exitstack(_emit)

_NC_CACHE = None


def _build():
    global _NC_CACHE
    if _NC_CACHE is not None:
        return _NC_CACHE
    nc = bacc.Bacc("TRN2", target_bir_lowering=False, debug=False)
    xT = nc.dram_tensor("xt", [D, N], F32R, kind="ExternalInput").ap()
    wq = nc.dram_tensor("wq", [D, DC], F32R, kind="ExternalInput").ap()
    wk = nc.dram_tensor("wk", [D, DC], F32R, kind="ExternalInput").ap()
    wv = nc.dram_tensor("wv", [D, DC], F32R, kind="ExternalInput").ap()
    wo = nc.dram_tensor("wo", [DC, D], F32R, kind="ExternalInput").ap()
    bq = nc.dram_tensor("bq", [DC], F32, kind="ExternalInput").ap()
    bk = nc.dram_tensor("bk", [DC], F32, kind="ExternalInput").ap()
    bv = nc.dram_tensor("bv", [DC], F32, kind="ExternalInput").ap()
    masks = nc.dram_tensor("masks", [NQB, P, QW], BF16, kind="ExternalInput").ap()
    out = nc.dram_tensor("out", [N, D], F32, kind="ExternalOutput").ap()
    with tile.TileContext(nc) as tc:
        _emit_wrapped(tc, xT, wq, wk, wv, wo, bq, bk, bv, masks, out)
    nc.compile()
    _NC_CACHE = nc
    return nc


def _make_masks():
    # masks[j] applies to the S^T tile whose key block sits j query-blocks
    # into the diagonal 512-wide region: [keys(128) x queries(512)].
    m = np.zeros((NQB, P, QW), np.float32)
    tri = np.triu(np.ones((P, P), np.float32))  # key <= query kept
    for j in range(NQB):
        for i in range(NQB):
            if i > j:
                m[j, :, i * P:(i + 1) * P] = 1.0
            elif i == j:
                m[j, :, i * P:(i + 1) * P] = tri
    return m.astype(ml_dtypes.bfloat16)


def _in_maps(x, Wq, bq, Wk, bk, Wv, bv, Wo):
    masks = _make_masks()
    maps = []
    for b in range(B):
        xt_b = np.ascontiguousarray(np.asarray(x[b]).T)
        for g in range(GROUPS):
            sl = slice(g * DC, (g + 1) * DC)
            maps.append({
                "xt": xt_b,
                "wq": np.ascontiguousarray(Wq[:, sl]),
                "wk": np.ascontiguousarray(Wk[:, sl]),
                "wv": np.ascontiguousarray(Wv[:, sl]),
                "wo": np.ascontiguousarray(Wo[sl, :]),
                "bq": np.ascontiguousarray(bq[sl]),
                "bk": np.ascontiguousarray(bk[sl]),
                "bv": np.ascontiguousarray(bv[sl]),
                "masks": masks,
            })
    return maps


def run(inputs, trace=False, tmpdir=None):
    """Build+run on 8 cores. Returns (out [B,N,D] f32, BassKernelResults)."""
    x = np.asarray(inputs["x"], np.float32)
    args = [np.asarray(inputs[k], np.float32) for k in
            ("Wq", "bq", "Wk", "bk", "Wv", "bv", "Wo")]
    bo = np.asarray(inputs["bo"], np.float32)
    nc = _build()
    maps = _in_maps(x, *args)
    if trace:
        bass_utils.upload_artifacts = lambda d: d
    res = bass_utils.run_bass_kernel_spmd(
        nc, maps, core_ids=list(range(8)), trace=trace, tmpdir=tmpdir)
    out = np.empty((B, N, D), np.float32)
    for b in range(B):
        out[b] = res.results[2 * b]["out"] + res.results[2 * b + 1]["out"] + bo
    return out, res


def kernel(**inputs):
    out, _ = run(inputs)
    return out


# revision 20
# speedup vs baseline: 1.8066x; 1.1453x over previous
"""Multi-head causal self-attention (B=4, N=2048, D=1024, H=16) on 8 TRN2 cores.

Sharding: 8 cores = 4 batches x 2 head-groups (8 heads / 512 dims each).
Per core (batch b, group g):
  - QKV projections computed in transposed layout (dims on partitions):
      Q^T, K^T = W^T-chunks (lhsT) x x^T (rhs), accumulated over 8 din chunks.
      V computed in natural [token, dv] layout (lhsT = x^T chunk).
  - Attention computed as S^T tiles [keys(128) x queries(512)] so that
    exp(S) feeds the P^T.V matmul directly (contraction over keys on
    partitions, no transposes anywhere). Softmax denominators come from a
    ones-column appended to V (row HD of the PV accumulator); normalization
    is deferred and batched per strip. Causal masking = skip blocks above
    the diagonal + multiply diagonal-region tiles by precomputed 0/1 masks
    after exp. No max-subtraction: scores are ~N(0,1) after the 1/sqrt(hd)
    scale, exp is safe in fp32.
  - O-projection partial: attnT (lhsT) x Wo-slice (rhs) -> [2048, 1024]
    partial output per core; host sums the two group partials per batch.

Dtypes: scores path float32r (fp32 storage, ~tf32 matmul precision, full PE
rate); P/V path bfloat16 (probs in [0,1], V ~N(0,1)).
"""

import numpy as np
import ml_dtypes

import concourse.bass as bass
import concourse.tile as tile
from concourse import bacc, mybir
from concourse import bass_utils
from concourse._compat import with_exitstack
from concourse.bass import ts, ds

B, N, D, H, HD = 4, 2048, 1024, 16, 64
GROUPS = 2              # head groups (cores per batch)
DC = D // GROUPS        # 512 dims per core
HPC = H // GROUPS       # 8 heads per core
P = 128
QW = 512                # query strip width / matmul free dim
NDIN = D // P           # 8 contraction chunks for QKV
NSTRIP = DC // P        # 4 dq strips per core (2 heads each)
NTT = N // P            # 16 token tiles
NTS = N // QW           # 4 token strips
NQB = QW // P           # 4 query blocks per strip

F32 = mybir.dt.float32
F32R = mybir.dt.float32r
BF16 = mybir.dt.bfloat16


def _emit(ctx, tc, xT, wq, wk, wv, wo, bq, bk, bv, masks, out):
    nc = tc.nc
    EXP = mybir.ActivationFunctionType.Exp

    const = ctx.enter_context(tc.tile_pool(name="const", bufs=1))
    p_mm = ctx.enter_context(tc.tile_pool(name="p_mm", bufs=2, space="PSUM"))
    p_pt = ctx.enter_context(tc.tile_pool(name="p_pt", bufs=3))
    p_small = ctx.enter_context(tc.tile_pool(name="p_small", bufs=2))
    p_dram = ctx.enter_context(tc.tile_pool(name="p_dram", bufs=2, space="DRAM"))

    # constants: masks [128, 4, 512], per-strip biases [128, 4], bv broadcast
    maskt = const.tile([P, NQB, QW], BF16)
    nc.sync.dma_start(out=maskt, in_=masks.rearrange("m p q -> p m q"))
    bqt = const.tile([P, NSTRIP], F32)
    nc.sync.dma_start(out=bqt, in_=bq.rearrange("(s p) -> p s", p=P))
    bkt = const.tile([P, NSTRIP], F32)
    nc.sync.dma_start(out=bkt, in_=bk.rearrange("(s p) -> p s", p=P))
    bvb = const.tile([P, DC], F32)
    nc.sync.dma_start(out=bvb, in_=bv.unsqueeze(0).partition_broadcast(P))

    # persistent per-batch tensors
    attnT = const.tile([P, NSTRIP, N], BF16)                # normalized attn^T
    vplus = const.tile([P, NTT, HPC, HD + 1], BF16)         # V | ones column
    # memset on an f32r/bf16 matmul-input tile is invalid ISA; write the ones
    # column via a DVE copy from an f32 staging tile (a valid rounding producer)
    ones_f32 = const.tile([P, NTT * HPC], F32)
    nc.vector.memset(ones_f32, 1.0)
    nc.vector.tensor_copy(
        out=vplus[:, :, :, HD:HD + 1],
        in_=ones_f32.rearrange("p (a b) -> p a b", b=HPC).unsqueeze(3),
    )

    # Wo loaded up-front so phase C never waits on its DMA
    wor = wo.rearrange("(c p) f -> c p f", p=P)
    wot = const.tile([P, NSTRIP, D], BF16)
    for c in range(NSTRIP):
        nc.sync.dma_start(out=wot[:, c, :], in_=wor[c])

    wqr = wq.rearrange("(c p) f -> c p f", p=P)
    wkr = wk.rearrange("(c p) f -> c p f", p=P)
    wvr = wv.rearrange("(c p) f -> c p f", p=P)
    xTr = xT.rearrange("(c p) n -> c p n", p=P)

    with tc.tile_pool(name="p_xt", bufs=1) as p_xt:
        xt = p_xt.tile([P, NDIN, N], BF16)          # x^T resident, 64KB/part
        # token-strip-major loads so early-strip compute can start ASAP
        for t in range(NTS):
            for c in range(NDIN):
                nc.sync.dma_start(
                    out=xt[:, c, ts(t, QW)], in_=xTr[c, :, ts(t, QW)])

        with (
            tc.tile_pool(name="p_w", bufs=2) as p_w,
            tc.tile_pool(name="p_qk", bufs=2) as p_qk,
            tc.tile_pool(name="p_st", bufs=2, space="PSUM") as p_st,
            tc.tile_pool(name="p_pv", bufs=2, space="PSUM") as p_pv,
        ):
            def attn_group(s, h2, qs, qts, kts, sums_sb):
                """S^T/exp/PV for one (head, query strip); kc-paired."""
                po = h2 * HD
                h = 2 * s + h2
                nkc = NQB * qs + NQB          # causal: key blocks 0..nkc-1
                npair = nkc // 2
                pvp = p_pv.tile([HD + 1, QW], F32, tag="pv", name="pvp")
                pts = {}
                LOOKP = 1
                for ip in range(npair + LOOKP):
                    if ip < npair:
                        pst = p_st.tile([P, 2, QW], F32, tag="st", name="pst")
                        for j2 in range(2):
                            kc = 2 * ip + j2
                            nc.tensor.matmul(
                                pst[:, j2, :],
                                lhsT=kts[po:po + HD, ts(kc, P)],
                                rhs=qts[po:po + HD, ts(qs, QW)],
                                start=True, stop=True,
                            )
                        pt = p_pt.tile([P, 2, QW], BF16, tag="pt", name="pt")
                        nc.scalar.activation(out=pt, in_=pst, func=EXP, scale=0.125)
                        for j2 in range(2):
                            kc = 2 * ip + j2
                            if kc >= NQB * qs:
                                nc.vector.tensor_mul(
                                    pt[:, j2, :], pt[:, j2, :],
                                    maskt[:, kc - NQB * qs, :])
                        pts[ip] = pt
                    if ip >= LOOKP:
                        jp = ip - LOOKP
                        pt = pts.pop(jp)
                        for j2 in range(2):
                            kc = 2 * jp + j2
                            nc.tensor.matmul(
                                pvp, lhsT=vplus[:, kc, h, :], rhs=pt[:, j2, :],
                                start=(kc == 0), stop=(kc == nkc - 1),
                            )
                nc.vector.tensor_copy(
                    out=sums_sb[32 * qs:32 * qs + 1, h2, :],
                    in_=pvp[HD:HD + 1, :])
                nc.vector.tensor_copy(
                    out=attnT[po:po + HD, s, ts(qs, QW)], in_=pvp[0:HD, :])

            def normalize_h2(s, h2, sums_sb):
                """Batched softmax normalization for one head (4 query strips)."""
                po = h2 * HD
                recip_sb = p_small.tile([P, QW], F32, tag="recip", name="recip_sb")
                nc.vector.reciprocal(out=recip_sb, in_=sums_sb[:, h2, :])
                # broadcast across partitions via a DRAM round-trip
                # (SBUF-source partition-broadcast DMA is rejected)
                recip_d = p_dram.tile([NTS, QW], F32, tag="recipd", name="recip_d")
                nc.sync.dma_start(
                    out=recip_d,
                    in_=recip_sb.rearrange("(a b) f -> a b f", b=32)[:, 0, :])
                for qs in range(NTS):
                    # full-128-partition broadcast so rb[po:po+HD] shares
                    # the base partition with the attnT slice (DVE rule)
                    rb = p_small.tile([P, QW], F32, tag="rb", bufs=3, name="rb")
                    nc.sync.dma_start(
                        out=rb,
                        in_=recip_d[qs, :].unsqueeze(0).partition_broadcast(P))
                    sl = attnT[po:po + HD, s, ts(qs, QW)]
                    nc.vector.tensor_mul(out=sl, in0=sl, in1=rb[po:po + HD, :])

            for s in range(NSTRIP):
                wqs = p_w.tile([P, NDIN, P], BF16, tag="wq")
                wks = p_w.tile([P, NDIN, P], BF16, tag="wk")
                for c in range(NDIN):
                    nc.sync.dma_start(out=wqs[:, c, :], in_=wqr[c, :, ts(s, P)])
                    nc.sync.dma_start(out=wks[:, c, :], in_=wkr[c, :, ts(s, P)])
                qts = p_qk.tile([P, N], BF16, tag="qt")
                kts = p_qk.tile([P, N], BF16, tag="kt")
                # sums rows at partition offsets {0,32,64,96} x 2 col blocks
                # (DVE partition offsets must be 32-aligned); unused rows are
                # memset to 1.0 so the batched reciprocal stays finite
                sums_sb = p_small.tile([P, 2, QW], F32, tag="sums")
                nc.vector.memset(sums_sb, 1.0)
                for t in range(NTS):
                    psq = p_mm.tile([P, QW], F32, tag="mm", name="psq")
                    for c in range(NDIN):
                        nc.tensor.matmul(
                            psq, lhsT=wqs[:, c, :], rhs=xt[:, c, ts(t, QW)],
                            start=(c == 0), stop=(c == NDIN - 1),
                        )
                    nc.vector.tensor_scalar_add(
                        out=qts[:, ts(t, QW)], in0=psq, scalar1=bqt[:, s:s + 1])
                    psk = p_mm.tile([P, QW], F32, tag="mm", name="psk")
                    for c in range(NDIN):
                        nc.tensor.matmul(
                            psk, lhsT=wks[:, c, :], rhs=xt[:, c, ts(t, QW)],
                            start=(c == 0), stop=(c == NDIN - 1),
                        )
                    nc.vector.tensor_scalar_add(
                        out=kts[:, ts(t, QW)], in0=psk, scalar1=bkt[:, s:s + 1])

                    if s == 0 and t == 0:
                        # V = x @ Wv + bv for all heads, nested here so the
                        # strip-0 attention (which needs early V tiles) can
                        # start as soon as possible
                        with tc.tile_pool(name="p_wv", bufs=1) as p_wv:
                            wvt = p_wv.tile([P, NDIN, DC], BF16)
                            for c in range(NDIN):
                                nc.sync.dma_start(out=wvt[:, c, :], in_=wvr[c])
                            for tt in range(NTT):
                                psv = p_mm.tile([P, DC], F32, tag="mm", name="psv")
                                for c in range(NDIN):
                                    nc.tensor.matmul(
                                        psv, lhsT=xt[:, c, ts(tt, P)],
                                        rhs=wvt[:, c, :],
                                        start=(c == 0), stop=(c == NDIN - 1),
                                    )
                                nc.vector.tensor_add(
                                    out=vplus[:, tt, :, 0:HD],
                                    in0=psv.rearrange("p (h d) -> p h d", d=HD),
                                    in1=bvb.rearrange("p (h d) -> p h d", d=HD),
                                )

                    if s == 0:
                        # strip 0: interleave attention with projections so
                        # compute starts before all x^T strips have landed
                        attn_group(s, 0, t, qts, kts, sums_sb)
                        attn_group(s, 1, t, qts, kts, sums_sb)
                if s > 0:
                    # head-major so h2=0's normalization overlaps h2=1's attention
                    for qs in range(NTS):
                        attn_group(s, 0, qs, qts, kts, sums_sb)
                    normalize_h2(s, 0, sums_sb)
                    for qs in range(NTS):
                        attn_group(s, 1, qs, qts, kts, sums_sb)
                    normalize_h2(s, 1, sums_sb)

                if s == 0:
                    normalize_h2(s, 0, sums_sb)
                    normalize_h2(s, 1, sums_sb)

    # ---- phase C: partial output = attnT^T @ Wo_slice ----
    with (
        tc.tile_pool(name="p_osb", bufs=3) as p_osb,
        tc.tile_pool(name="p_c", bufs=4, space="PSUM") as p_c,
    ):
        for tt in range(NTT):
            osb = p_osb.tile([P, D], F32, tag="osb")
            for half in range(2):
                pso = p_c.tile([P, QW], F32, tag="c", name="pso")
                for c in range(NSTRIP):
                    nc.tensor.matmul(
                        pso, lhsT=attnT[:, c, ts(tt, P)],
                        rhs=wot[:, c, ds(half * QW, QW)],
                        start=(c == 0), stop=(c == NSTRIP - 1),
                    )
                nc.vector.tensor_copy(out=osb[:, ds(half * QW, QW)], in_=pso)
            nc.sync.dma_start(out=out[ts(tt, P), :], in_=osb)


_emit_wrapped = with_exitstack(_emit)

_NC_CACHE = None


def _build():
    global _NC_CACHE
    if _NC_CACHE is not None:
        return _NC_CACHE
    nc = bacc.Bacc("TRN2", target_bir_lowering=False, debug=False)
    xT = nc.dram_tensor("xt", [D, N], BF16, kind="ExternalInput").ap()
    wq = nc.dram_tensor("wq", [D, DC], BF16, kind="ExternalInput").ap()
    wk = nc.dram_tensor("wk", [D, DC], BF16, kind="ExternalInput").ap()
    wv = nc.dram_tensor("wv", [D, DC], BF16, kind="ExternalInput").ap()
    wo = nc.dram_tensor("wo", [DC, D], BF16, kind="ExternalInput").ap()
    bq = nc.dram_tensor("bq", [DC], F32, kind="ExternalInput").ap()
    bk = nc.dram_tensor("bk", [DC], F32, kind="ExternalInput").ap()
    bv = nc.dram_tensor("bv", [DC], F32, kind="ExternalInput").ap()
    masks = nc.dram_tensor("masks", [NQB, P, QW], BF16, kind="ExternalInput").ap()
    out = nc.dram_tensor("out", [N, D], F32, kind="ExternalOutput").ap()
    with tile.TileContext(nc) as tc:
        _emit_wrapped(tc, xT, wq, wk, wv, wo, bq, bk, bv, masks, out)
    nc.compile()
    _NC_CACHE = nc
    return nc


def _make_masks():
    # masks[j] applies to the S^T tile whose key block sits j query-blocks
    # into the diagonal 512-wide region: [keys(128) x queries(512)].
    m = np.zeros((NQB, P, QW), np.float32)
    tri = np.triu(np.ones((P, P), np.float32))  # key <= query kept
    for j in range(NQB):
        for i in range(NQB):
            if i > j:
                m[j, :, i * P:(i + 1) * P] = 1.0
            elif i == j:
                m[j, :, i * P:(i + 1) * P] = tri
    return m.astype(ml_dtypes.bfloat16)


def _in_maps(x, Wq, bq, Wk, bk, Wv, bv, Wo):
    masks = _make_masks()
    maps = []
    for b in range(B):
        xt_b = np.ascontiguousarray(np.asarray(x[b]).T)
        for g in range(GROUPS):
            sl = slice(g * DC, (g + 1) * DC)
            bf = ml_dtypes.bfloat16
            maps.append({
                "xt": xt_b.astype(bf),
                "wq": np.ascontiguousarray(Wq[:, sl]).astype(bf),
                "wk": np.ascontiguousarray(Wk[:, sl]).astype(bf),
                "wv": np.ascontiguousarray(Wv[:, sl]).astype(bf),
                "wo": np.ascontiguousarray(Wo[sl, :]).astype(bf),
                "bq": np.ascontiguousarray(bq[sl]),
                "bk": np.ascontiguousarray(bk[sl]),
                "bv": np.ascontiguousarray(bv[sl]),
                "masks": masks,
            })
    return maps


def run(inputs, trace=False, tmpdir=None):
    """Build+run on 8 cores. Returns (out [B,N,D] f32, BassKernelResults)."""
    x = np.asarray(inputs["x"], np.float32)
    args = [np.asarray(inputs[k], np.float32) for k in
            ("Wq", "bq", "Wk", "bk", "Wv", "bv", "Wo")]
    bo = np.asarray(inputs["bo"], np.float32)
    nc = _build()
    maps = _in_maps(x, *args)
    if trace:
        bass_utils.upload_artifacts = lambda d: d
    res = bass_utils.run_bass_kernel_spmd(
        nc, maps, core_ids=list(range(8)), trace=trace, tmpdir=tmpdir)
    out = np.empty((B, N, D), np.float32)
    for b in range(B):
        out[b] = res.results[2 * b]["out"] + res.results[2 * b + 1]["out"] + bo
    return out, res


def kernel(**inputs):
    out, _ = run(inputs)
    return out


# revision 21
# speedup vs baseline: 1.8498x; 1.0239x over previous
"""Multi-head causal self-attention (B=4, N=2048, D=1024, H=16) on 8 TRN2 cores.

Sharding: 8 cores = 4 batches x 2 head-groups (8 heads / 512 dims each).
Per core (batch b, group g):
  - QKV projections computed in transposed layout (dims on partitions):
      Q^T, K^T = W^T-chunks (lhsT) x x^T (rhs), accumulated over 8 din chunks.
      V computed in natural [token, dv] layout (lhsT = x^T chunk).
  - Attention computed as S^T tiles [keys(128) x queries(512)] so that
    exp(S) feeds the P^T.V matmul directly (contraction over keys on
    partitions, no transposes anywhere). Softmax denominators come from a
    ones-column appended to V (row HD of the PV accumulator); normalization
    is deferred and batched per strip. Causal masking = skip blocks above
    the diagonal + multiply diagonal-region tiles by precomputed 0/1 masks
    after exp. No max-subtraction: scores are ~N(0,1) after the 1/sqrt(hd)
    scale, exp is safe in fp32.
  - O-projection partial: attnT (lhsT) x Wo-slice (rhs) -> [2048, 1024]
    partial output per core; host sums the two group partials per batch.

Dtypes: scores path float32r (fp32 storage, ~tf32 matmul precision, full PE
rate); P/V path bfloat16 (probs in [0,1], V ~N(0,1)).
"""

import numpy as np
import ml_dtypes

import concourse.bass as bass
import concourse.tile as tile
from concourse import bacc, mybir
from concourse import bass_utils
from concourse._compat import with_exitstack
from concourse.bass import ts, ds

B, N, D, H, HD = 4, 2048, 1024, 16, 64
GROUPS = 2              # head groups (cores per batch)
DC = D // GROUPS        # 512 dims per core
HPC = H // GROUPS       # 8 heads per core
P = 128
QW = 512                # query strip width / matmul free dim
NDIN = D // P           # 8 contraction chunks for QKV
NSTRIP = DC // P        # 4 dq strips per core (2 heads each)
NTT = N // P            # 16 token tiles
NTS = N // QW           # 4 token strips
NQB = QW // P           # 4 query blocks per strip

F32 = mybir.dt.float32
F32R = mybir.dt.float32r
BF16 = mybir.dt.bfloat16


def _emit(ctx, tc, xT, wq, wk, wv, wo, bq, bk, bv, masks, out):
    nc = tc.nc
    EXP = mybir.ActivationFunctionType.Exp

    const = ctx.enter_context(tc.tile_pool(name="const", bufs=1))
    p_mm = ctx.enter_context(tc.tile_pool(name="p_mm", bufs=2, space="PSUM"))
    p_pt = ctx.enter_context(tc.tile_pool(name="p_pt", bufs=3))
    p_small = ctx.enter_context(tc.tile_pool(name="p_small", bufs=2))
    p_dram = ctx.enter_context(tc.tile_pool(name="p_dram", bufs=2, space="DRAM"))

    # constants on the GpSimd (SWDGE) queue so they don't serialize with the
    # x^T stream on the sync (HWDGE) queue. maskt = one triangular 0/1 tile.
    maskt = const.tile([P, P], BF16)
    nc.gpsimd.dma_start(out=maskt, in_=masks)
    bqt = const.tile([P, NSTRIP], F32)
    nc.gpsimd.dma_start(out=bqt, in_=bq.rearrange("(s p) -> p s", p=P))
    bkt = const.tile([P, NSTRIP], F32)
    nc.gpsimd.dma_start(out=bkt, in_=bk.rearrange("(s p) -> p s", p=P))
    bvb = const.tile([P, DC], F32)
    nc.gpsimd.dma_start(out=bvb, in_=bv.unsqueeze(0).partition_broadcast(P))

    # persistent per-batch tensors
    attnT = const.tile([P, NSTRIP, N], BF16)                # normalized attn^T
    vplus = const.tile([P, NTT, HPC, HD + 1], BF16)         # V | ones column
    # memset on an f32r/bf16 matmul-input tile is invalid ISA; write the ones
    # column via a DVE copy from an f32 staging tile (a valid rounding producer)
    ones_f32 = const.tile([P, NTT * HPC], F32)
    nc.vector.memset(ones_f32, 1.0)
    nc.vector.tensor_copy(
        out=vplus[:, :, :, HD:HD + 1],
        in_=ones_f32.rearrange("p (a b) -> p a b", b=HPC).unsqueeze(3),
    )

    # Wo loaded up-front so phase C never waits on its DMA
    wor = wo.rearrange("(c p) f -> c p f", p=P)
    wot = const.tile([P, NSTRIP, D], BF16)
    for c in range(NSTRIP):
        nc.gpsimd.dma_start(out=wot[:, c, :], in_=wor[c])

    wqr = wq.rearrange("(c p) f -> c p f", p=P)
    wkr = wk.rearrange("(c p) f -> c p f", p=P)
    wvr = wv.rearrange("(c p) f -> c p f", p=P)
    xTr = xT.rearrange("(c p) n -> c p n", p=P)

    with tc.tile_pool(name="p_xt", bufs=1) as p_xt:
        xt = p_xt.tile([P, NDIN, N], BF16)          # x^T resident, 64KB/part
        # token-strip-major loads so early-strip compute can start ASAP
        for t in range(NTS):
            for c in range(NDIN):
                nc.sync.dma_start(
                    out=xt[:, c, ts(t, QW)], in_=xTr[c, :, ts(t, QW)])

        with (
            tc.tile_pool(name="p_w", bufs=2) as p_w,
            tc.tile_pool(name="p_qk", bufs=2) as p_qk,
            tc.tile_pool(name="p_st", bufs=2, space="PSUM") as p_st,
            tc.tile_pool(name="p_pv", bufs=2, space="PSUM") as p_pv,
        ):
            def attn_group(s, h2, qs, qts, kts, sums_sb):
                """S^T/exp/PV for one (head, query strip).

                Work units: full-width kc pairs below the diagonal region,
                then two packed diagonal units with shrinking query widths
                (512+384 and 256+128) — queries before the key block are
                skipped entirely, the remaining 128-wide leading wedge of
                each unit gets the triangular mask.
                """
                po = h2 * HD
                h = 2 * s + h2
                nfull = NQB * qs             # unmasked key blocks 0..nfull-1
                nkc = nfull + NQB
                q0 = qs * QW
                pvp = p_pv.tile([HD + 1, QW], F32, tag="pv", name="pvp")

                units = []
                for ip in range(nfull // 2):
                    units.append(("full", ip))
                units.append(("diagA", None))
                units.append(("diagB", None))

                def emit_s(unit):
                    kind, ip = unit
                    if kind == "full":
                        pst = p_st.tile([P, 2, QW], F32, tag="st", name="pst")
                        for j2 in range(2):
                            kc = 2 * ip + j2
                            nc.tensor.matmul(
                                pst[:, j2, :],
                                lhsT=kts[po:po + HD, ts(kc, P)],
                                rhs=qts[po:po + HD, ts(qs, QW)],
                                start=True, stop=True,
                            )
                        pt = p_pt.tile([P, 2, QW], BF16, tag="pt", name="pt")
                        nc.scalar.activation(out=pt, in_=pst, func=EXP, scale=0.125)
                        return pt
                    if kind == "diagA":
                        # j=0: kc=nfull,   queries [0:512), tri on cols 0:128
                        # j=1: kc=nfull+1, queries [128:512), tri on cols 0:128
                        pst = p_st.tile([P, 2, QW], F32, tag="st", name="pst")
                        nc.tensor.matmul(
                            pst[:, 0, :],
                            lhsT=kts[po:po + HD, ts(nfull, P)],
                            rhs=qts[po:po + HD, ts(qs, QW)],
                            start=True, stop=True,
                        )
                        nc.tensor.matmul(
                            pst[:, 1, 0:3 * P],
                            lhsT=kts[po:po + HD, ts(nfull + 1, P)],
                            rhs=qts[po:po + HD, ds(q0 + P, 3 * P)],
                            start=True, stop=True,
                        )
                        pt = p_pt.tile([P, 2, QW], BF16, tag="pt", name="pt")
                        nc.scalar.activation(out=pt, in_=pst, func=EXP, scale=0.125)
                        nc.vector.tensor_mul(pt[:, 0, 0:P], pt[:, 0, 0:P], maskt)
                        nc.vector.tensor_mul(pt[:, 1, 0:P], pt[:, 1, 0:P], maskt)
                        return pt
                    # diagB: j=2: kc=nfull+2, queries [256:512) at cols 0:256;
                    #        j=3: kc=nfull+3, queries [384:512) at cols 256:384
                    pst = p_st.tile([P, QW], F32, tag="st", name="pst")
                    nc.tensor.matmul(
                        pst[:, 0:2 * P],
                        lhsT=kts[po:po + HD, ts(nfull + 2, P)],
                        rhs=qts[po:po + HD, ds(q0 + 2 * P, 2 * P)],
                        start=True, stop=True,
                    )
                    nc.tensor.matmul(
                        pst[:, 2 * P:3 * P],
                        lhsT=kts[po:po + HD, ts(nfull + 3, P)],
                        rhs=qts[po:po + HD, ds(q0 + 3 * P, P)],
                        start=True, stop=True,
                    )
                    pt = p_pt.tile([P, QW], BF16, tag="pt", name="pt")
                    nc.scalar.activation(out=pt, in_=pst, func=EXP, scale=0.125)
                    nc.vector.tensor_mul(pt[:, 0:P], pt[:, 0:P], maskt)
                    nc.vector.tensor_mul(pt[:, 2 * P:3 * P], pt[:, 2 * P:3 * P], maskt)
                    return pt

                def emit_pv(unit, pt):
                    kind, ip = unit
                    if kind == "full":
                        for j2 in range(2):
                            kc = 2 * ip + j2
                            nc.tensor.matmul(
                                pvp, lhsT=vplus[:, kc, h, :], rhs=pt[:, j2, :],
                                start=(kc == 0), stop=False,
                            )
                    elif kind == "diagA":
                        nc.tensor.matmul(
                            pvp, lhsT=vplus[:, nfull, h, :], rhs=pt[:, 0, :],
                            start=(nfull == 0), stop=False,
                        )
                        nc.tensor.matmul(
                            pvp[:, P:4 * P], lhsT=vplus[:, nfull + 1, h, :],
                            rhs=pt[:, 1, 0:3 * P], start=False, stop=False,
                        )
                    else:
                        nc.tensor.matmul(
                            pvp[:, 2 * P:4 * P], lhsT=vplus[:, nfull + 2, h, :],
                            rhs=pt[:, 0:2 * P], start=False, stop=False,
                        )
                        nc.tensor.matmul(
                            pvp[:, 3 * P:4 * P], lhsT=vplus[:, nfull + 3, h, :],
                            rhs=pt[:, 2 * P:3 * P], start=False, stop=True,
                        )

                LOOKP = 1
                pts = {}
                for i in range(len(units) + LOOKP):
                    if i < len(units):
                        pts[i] = emit_s(units[i])
                    if i >= LOOKP:
                        j = i - LOOKP
                        emit_pv(units[j], pts.pop(j))
                nc.vector.tensor_copy(
                    out=sums_sb[32 * qs:32 * qs + 1, h2, :],
                    in_=pvp[HD:HD + 1, :])
                nc.vector.tensor_copy(
                    out=attnT[po:po + HD, s, ts(qs, QW)], in_=pvp[0:HD, :])

            def normalize_h2(s, h2, sums_sb):
                """Batched softmax normalization for one head (4 query strips)."""
                po = h2 * HD
                recip_sb = p_small.tile([P, QW], F32, tag="recip", name="recip_sb")
                nc.vector.reciprocal(out=recip_sb, in_=sums_sb[:, h2, :])
                # broadcast across partitions via a DRAM round-trip
                # (SBUF-source partition-broadcast DMA is rejected)
                recip_d = p_dram.tile([NTS, QW], F32, tag="recipd", name="recip_d")
                nc.sync.dma_start(
                    out=recip_d,
                    in_=recip_sb.rearrange("(a b) f -> a b f", b=32)[:, 0, :])
                for qs in range(NTS):
                    # full-128-partition broadcast so rb[po:po+HD] shares
                    # the base partition with the attnT slice (DVE rule)
                    rb = p_small.tile([P, QW], F32, tag="rb", bufs=3, name="rb")
                    nc.sync.dma_start(
                        out=rb,
                        in_=recip_d[qs, :].unsqueeze(0).partition_broadcast(P))
                    sl = attnT[po:po + HD, s, ts(qs, QW)]
                    nc.vector.tensor_mul(out=sl, in0=sl, in1=rb[po:po + HD, :])

            for s in range(NSTRIP):
                wqs = p_w.tile([P, NDIN, P], BF16, tag="wq")
                wks = p_w.tile([P, NDIN, P], BF16, tag="wk")
                for c in range(NDIN):
                    nc.gpsimd.dma_start(out=wqs[:, c, :], in_=wqr[c, :, ts(s, P)])
                    nc.gpsimd.dma_start(out=wks[:, c, :], in_=wkr[c, :, ts(s, P)])
                qts = p_qk.tile([P, N], BF16, tag="qt")
                kts = p_qk.tile([P, N], BF16, tag="kt")
                # sums rows at partition offsets {0,32,64,96} x 2 col blocks
                # (DVE partition offsets must be 32-aligned); unused rows are
                # memset to 1.0 so the batched reciprocal stays finite
                sums_sb = p_small.tile([P, 2, QW], F32, tag="sums")
                nc.vector.memset(sums_sb, 1.0)
                for t in range(NTS):
                    psq = p_mm.tile([P, QW], F32, tag="mm", name="psq")
                    for c in range(NDIN):
                        nc.tensor.matmul(
                            psq, lhsT=wqs[:, c, :], rhs=xt[:, c, ts(t, QW)],
                            start=(c == 0), stop=(c == NDIN - 1),
                        )
                    nc.vector.tensor_scalar_add(
                        out=qts[:, ts(t, QW)], in0=psq, scalar1=bqt[:, s:s + 1])
                    psk = p_mm.tile([P, QW], F32, tag="mm", name="psk")
                    for c in range(NDIN):
                        nc.tensor.matmul(
                            psk, lhsT=wks[:, c, :], rhs=xt[:, c, ts(t, QW)],
                            start=(c == 0), stop=(c == NDIN - 1),
                        )
                    nc.vector.tensor_scalar_add(
                        out=kts[:, ts(t, QW)], in0=psk, scalar1=bkt[:, s:s + 1])

                    if s == 0 and t == 0:
                        # V = x @ Wv + bv for all heads, nested here so the
                        # strip-0 attention (which needs early V tiles) can
                        # start as soon as possible
                        with tc.tile_pool(name="p_wv", bufs=1) as p_wv:
                            wvt = p_wv.tile([P, NDIN, DC], BF16)
                            for c in range(NDIN):
                                nc.gpsimd.dma_start(out=wvt[:, c, :], in_=wvr[c])
                            for tt in range(NTT):
                                psv = p_mm.tile([P, DC], F32, tag="mm", name="psv")
                                for c in range(NDIN):
                                    nc.tensor.matmul(
                                        psv, lhsT=xt[:, c, ts(tt, P)],
                                        rhs=wvt[:, c, :],
                                        start=(c == 0), stop=(c == NDIN - 1),
                                    )
                                nc.vector.tensor_add(
                                    out=vplus[:, tt, :, 0:HD],
                                    in0=psv.rearrange("p (h d) -> p h d", d=HD),
                                    in1=bvb.rearrange("p (h d) -> p h d", d=HD),
                                )

                    if s == 0:
                        # strip 0: interleave attention with projections so
                        # compute starts before all x^T strips have landed
                        attn_group(s, 0, t, qts, kts, sums_sb)
                        attn_group(s, 1, t, qts, kts, sums_sb)
                if s > 0:
                    # head-major so h2=0's normalization overlaps h2=1's attention
                    for qs in range(NTS):
                        attn_group(s, 0, qs, qts, kts, sums_sb)
                    normalize_h2(s, 0, sums_sb)
                    for qs in range(NTS):
                        attn_group(s, 1, qs, qts, kts, sums_sb)
                    normalize_h2(s, 1, sums_sb)

                if s == 0:
                    normalize_h2(s, 0, sums_sb)
                    normalize_h2(s, 1, sums_sb)

    # ---- phase C: partial output = attnT^T @ Wo_slice ----
    with (
        tc.tile_pool(name="p_osb", bufs=3) as p_osb,
        tc.tile_pool(name="p_c", bufs=4, space="PSUM") as p_c,
    ):
        for tt in range(NTT):
            osb = p_osb.tile([P, D], F32, tag="osb")
            for half in range(2):
                pso = p_c.tile([P, QW], F32, tag="c", name="pso")
                for c in range(NSTRIP):
                    nc.tensor.matmul(
                        pso, lhsT=attnT[:, c, ts(tt, P)],
                        rhs=wot[:, c, ds(half * QW, QW)],
                        start=(c == 0), stop=(c == NSTRIP - 1),
                    )
                nc.vector.tensor_copy(out=osb[:, ds(half * QW, QW)], in_=pso)
            nc.sync.dma_start(out=out[ts(tt, P), :], in_=osb)


_emit_wrapped = with_exitstack(_emit)

_NC_CACHE = None


def _build():
    global _NC_CACHE
    if _NC_CACHE is not None:
        return _NC_CACHE
    nc = bacc.Bacc("TRN2", target_bir_lowering=False, debug=False)
    xT = nc.dram_tensor("xt", [D, N], BF16, kind="ExternalInput").ap()
    wq = nc.dram_tensor("wq", [D, DC], BF16, kind="ExternalInput").ap()
    wk = nc.dram_tensor("wk", [D, DC], BF16, kind="ExternalInput").ap()
    wv = nc.dram_tensor("wv", [D, DC], BF16, kind="ExternalInput").ap()
    wo = nc.dram_tensor("wo", [DC, D], BF16, kind="ExternalInput").ap()
    bq = nc.dram_tensor("bq", [DC], F32, kind="ExternalInput").ap()
    bk = nc.dram_tensor("bk", [DC], F32, kind="ExternalInput").ap()
    bv = nc.dram_tensor("bv", [DC], F32, kind="ExternalInput").ap()
    masks = nc.dram_tensor("masks", [P, P], BF16, kind="ExternalInput").ap()
    out = nc.dram_tensor("out", [N, D], F32, kind="ExternalOutput").ap()
    with tile.TileContext(nc) as tc:
        _emit_wrapped(tc, xT, wq, wk, wv, wo, bq, bk, bv, masks, out)
    nc.compile()
    _NC_CACHE = nc
    return nc


def _make_masks():
    # triangular 0/1 tile for the diagonal blocks of S^T: key <= query kept
    return np.triu(np.ones((P, P), np.float32)).astype(ml_dtypes.bfloat16)


def _in_maps(x, Wq, bq, Wk, bk, Wv, bv, Wo):
    masks = _make_masks()
    maps = []
    for b in range(B):
        xt_b = np.ascontiguousarray(np.asarray(x[b]).T)
        for g in range(GROUPS):
            sl = slice(g * DC, (g + 1) * DC)
            bf = ml_dtypes.bfloat16
            maps.append({
                "xt": xt_b.astype(bf),
                "wq": np.ascontiguousarray(Wq[:, sl]).astype(bf),
                "wk": np.ascontiguousarray(Wk[:, sl]).astype(bf),
                "wv": np.ascontiguousarray(Wv[:, sl]).astype(bf),
                "wo": np.ascontiguousarray(Wo[sl, :]).astype(bf),
                "bq": np.ascontiguousarray(bq[sl]),
                "bk": np.ascontiguousarray(bk[sl]),
                "bv": np.ascontiguousarray(bv[sl]),
                "masks": masks,
            })
    return maps


def run(inputs, trace=False, tmpdir=None):
    """Build+run on 8 cores. Returns (out [B,N,D] f32, BassKernelResults)."""
    x = np.asarray(inputs["x"], np.float32)
    args = [np.asarray(inputs[k], np.float32) for k in
            ("Wq", "bq", "Wk", "bk", "Wv", "bv", "Wo")]
    bo = np.asarray(inputs["bo"], np.float32)
    nc = _build()
    maps = _in_maps(x, *args)
    if trace:
        bass_utils.upload_artifacts = lambda d: d
    res = bass_utils.run_bass_kernel_spmd(
        nc, maps, core_ids=list(range(8)), trace=trace, tmpdir=tmpdir)
    out = np.empty((B, N, D), np.float32)
    for b in range(B):
        out[b] = res.results[2 * b]["out"] + res.results[2 * b + 1]["out"] + bo
    return out, res


def kernel(**inputs):
    out, _ = run(inputs)
    return out
